# revision 3
# baseline (speedup 1.0000x reference)
"""MoE-routing kernel for Trainium2 (8 NeuronCores, data-parallel over batch).

Problem (nn_DAWN_12979391168723):
  x [8,2048,1024] -> h = x@W + b [8,2048,64]
  logits = h @ normalize(neuron_emb).T   (only first 928 of 1184 neurons used)
  per-group softmax over neuron axis, importance-weighted sum over sequence
  -> per-batch routing weights; tiny top-k/sort/sparsify on [8, n] on host.

Sharding: batch b -> core b (8 cores). Each core computes w[b, 0:928] from
x[b] (8 MiB). Small weights (W, b, emb table) replicated per core.

Device layout notes:
  - x is fed pre-transposed per batch: xT [1024, 2048] so the contraction
    dim (d_model) lands on SBUF partitions for the tensor engine.
  - matmul1 produces hT [64, s] in PSUM: out = Wk[k,:].T @ xT[k,:]
  - matmul2: logits [128s, 928] = hT_chunk.T @ embT (emb pre-normalized,
    transposed on host: [64, 928]).
  - softmax: exp on ScalarE (PSUM->SBUF) with row-accum for the qk group;
    v/rel/val row-sums on VectorE. a = importance / Z per group.
  - reduce over s via matmul: w[g, n] = sum_s a[s, g] * E[s, n], PSUM
    accumulated across all 16 s-chunks.
"""

import numpy as np

import concourse.bacc as bacc
import concourse.mybir as mybir
import concourse.tile as tile
from concourse.bass_utils import run_bass_kernel_spmd

F32 = mybir.dt.float32
F32R = mybir.dt.float32r

# problem constants (hardcoded per spec)
B = 8
S = 2048
D = 1024
DS = 64
NT = 928                    # used neurons: 512 qk | 256 v | 128 rel | 32 val
N_CORES = 8
S_CHUNK = 512
N_SCHUNK = S // S_CHUNK     # 4
SUB = 128
N_SUB = S // SUB            # 16
KC = D // 128               # 8 contraction chunks

N_FQK, N_FV, N_REL, N_VAL = 512, 256, 128, 32
K_FQK, K_FV, K_REL, K_VAL = 64, 32, 16, 3

USE_F32R = True


def build_nc(use_f32r=USE_F32R):
    nc = bacc.Bacc("TRN2", target_bir_lowering=False, debug=False)

    # matmul operand dtype: float32r runs the PE at full rate (1 cyc/row for
    # free dim >= 256) vs plain float32's 4 cyc/row. The BIR verifier
    # requires every producer feeding an FP32r matmul to emit FP32r, so the
    # whole operand chain (DRAM params included) is typed with `mdt`.
    # np-side both map to float32.
    mdt = F32R if use_f32r else F32

    xT = nc.declare_dram_parameter("xT", [D, S], mdt, isOutput=False)
    Wk = nc.declare_dram_parameter("Wk", [128, KC, DS], mdt, isOutput=False)
    bcol = nc.declare_dram_parameter("bcol", [DS, 1], F32, isOutput=False)
    embT = nc.declare_dram_parameter("embT", [DS, NT], mdt, isOutput=False)
    impT = nc.declare_dram_parameter("impT", [128, N_SUB], F32, isOutput=False)
    wq_out = nc.declare_dram_parameter("wq", [1, 512], F32, isOutput=True)
    wvr_out = nc.declare_dram_parameter("wvr", [3, 416], F32, isOutput=True)

    def mm(ap):
        return ap

    Exp = mybir.ActivationFunctionType.Exp
    X = mybir.AxisListType.X
    ADD = mybir.AluOpType.add

    with tile.TileContext(nc) as tc:
        with (
            tc.tile_pool(name="const", bufs=1) as const_pool,
            tc.tile_pool(name="xt", bufs=16) as xt_pool,
            tc.tile_pool(name="h", bufs=4) as h_pool,
            tc.tile_pool(name="e", bufs=3) as e_pool,
            tc.tile_pool(name="stat", bufs=6) as s_pool,
            tc.tile_pool(name="out", bufs=1) as out_pool,
            tc.tile_pool(name="ph", bufs=2, space="PSUM") as ph_pool,
            tc.tile_pool(name="pl1", bufs=2, space="PSUM") as pl1_pool,
            tc.tile_pool(name="pl2", bufs=2, space="PSUM") as pl2_pool,
            tc.tile_pool(name="pw", bufs=1, space="PSUM") as pw_pool,
        ):
            wk_t = const_pool.tile([128, KC, DS], mdt, tag="wk")
            nc.sync.dma_start(wk_t[:], Wk[:])
            emb_t = const_pool.tile([DS, NT], mdt, tag="emb")
            nc.sync.dma_start(emb_t[:], embT[:])
            imp_t = const_pool.tile([128, N_SUB], F32, tag="imp")
            nc.sync.dma_start(imp_t[:], impT[:])
            b_t = const_pool.tile([DS, 1], F32, tag="b")
            nc.sync.dma_start(b_t[:], bcol[:])

            # routing-weight accumulators, live across the whole kernel
            pw1 = pw_pool.tile([1, 512], F32, tag="pw1")
            pw2 = pw_pool.tile([3, 416], F32, tag="pw2")

            for j in range(N_SCHUNK):
                ph = ph_pool.tile([DS, S_CHUNK], F32, tag="ph")
                for k in range(KC):
                    xt = xt_pool.tile([128, S_CHUNK], mdt, tag="xt")
                    nc.sync.dma_start(
                        xt[:], xT[k * 128:(k + 1) * 128,
                                  j * S_CHUNK:(j + 1) * S_CHUNK])
                    nc.tensor.matmul(ph[:], mm(wk_t[:, k, :]), mm(xt[:]),
                                     start=(k == 0), stop=(k == KC - 1))
                hj = h_pool.tile([DS, S_CHUNK], mdt, tag="h")
                # bias add fused into the PSUM->SBUF copy
                nc.vector.tensor_scalar_add(hj[:], ph[:], b_t[:])

                for tt in range(4):
                    t = j * 4 + tt
                    hs = hj[:, tt * SUB:(tt + 1) * SUB]        # [64, 128]
                    pl1 = pl1_pool.tile([SUB, 512], F32, tag="pl1")
                    pl2 = pl2_pool.tile([SUB, 416], F32, tag="pl2")
                    nc.tensor.matmul(pl1[:], mm(hs), mm(emb_t[:, 0:512]),
                                     start=True, stop=True)
                    nc.tensor.matmul(pl2[:], mm(hs), mm(emb_t[:, 512:NT]),
                                     start=True, stop=True)

                    et = e_pool.tile([SUB, NT], mdt, tag="e")
                    zt = s_pool.tile([SUB, 4], F32, tag="z")
                    rt = s_pool.tile([SUB, 4], F32, tag="r")
                    at = s_pool.tile([SUB, 4], mdt, tag="a")
                    # exp(logits); qk row-sum comes free via accum_out
                    nc.scalar.activation(et[:, 0:512], pl1[:], Exp,
                                         accum_out=zt[:, 0:1])
                    nc.scalar.activation(et[:, 512:NT], pl2[:], Exp)
                    nc.vector.tensor_reduce(zt[:, 1:2], et[:, 512:768],
                                            axis=X, op=ADD)
                    nc.vector.tensor_reduce(zt[:, 2:3], et[:, 768:896],
                                            axis=X, op=ADD)
                    nc.vector.tensor_reduce(zt[:, 3:4], et[:, 896:NT],
                                            axis=X, op=ADD)
                    nc.vector.reciprocal(rt[:], zt[:])
                    nc.vector.tensor_scalar_mul(at[:], rt[:],
                                                imp_t[:, t:t + 1])
                    nc.tensor.matmul(pw1[:], mm(at[:, 0:1]), mm(et[:, 0:512]),
                                     start=(t == 0), stop=(t == N_SUB - 1),
                                     skip_group_check=True)
                    nc.tensor.matmul(pw2[:], mm(at[:, 1:4]), mm(et[:, 512:NT]),
                                     start=(t == 0), stop=(t == N_SUB - 1),
                                     skip_group_check=True)

            wo1 = out_pool.tile([1, 512], F32, tag="wo1")
            wo2 = out_pool.tile([3, 416], F32, tag="wo2")
            nc.vector.tensor_copy(wo1[:], pw1[:])
            nc.vector.tensor_copy(wo2[:], pw2[:])
            nc.sync.dma_start(wq_out[:], wo1[:])
            nc.sync.dma_start(wvr_out[:], wo2[:])

    nc.compile()
    return nc


def host_prep(x, importance, W, b, neuron_emb):
    x = np.asarray(x, dtype=np.float32)
    importance = np.asarray(importance, dtype=np.float32)
    W = np.asarray(W, dtype=np.float32)
    b = np.asarray(b, dtype=np.float32)
    emb = np.asarray(neuron_emb, dtype=np.float32)

    xT = np.ascontiguousarray(x.transpose(0, 2, 1))               # [B, D, S]
    impT = np.ascontiguousarray(
        importance.reshape(B, N_SUB, 128).transpose(0, 2, 1))     # [B, 128, 16]
    emb_n = emb / np.linalg.norm(emb, axis=-1, keepdims=True)
    embT = np.ascontiguousarray(emb_n[:NT].T)                     # [64, 928]
    Wk = np.ascontiguousarray(
        W.reshape(KC, 128, DS).transpose(1, 0, 2))                # [128, 8, 64]
    bcol = np.ascontiguousarray(b.reshape(DS, 1))
    return xT, impT, embT, Wk, bcol


def make_in_maps(x, importance, W, b, neuron_emb):
    xT, impT, embT, Wk, bcol = host_prep(x, importance, W, b, neuron_emb)
    return [
        dict(xT=xT[i], impT=impT[i], embT=embT, Wk=Wk, bcol=bcol)
        for i in range(B)
    ]


def postprocess(w_qk, w_v, w_rel, w_val):
    def topk_idx(w, k):
        # match jax.lax.top_k tie-breaking (stable: lower index first)
        return np.argsort(-w, axis=-1, kind="stable")[:, :k]

    def sparsify(w, k):
        idx = topk_idx(w, k)
        out = np.zeros_like(w)
        rows = np.arange(w.shape[0])[:, None]
        out[rows, idx] = w[rows, idx]
        return out

    idx_qk = np.sort(topk_idx(w_qk, K_FQK), axis=-1).astype(np.int32)
    idx_v = np.sort(topk_idx(w_v, K_FV), axis=-1).astype(np.int32)
    rw_Q = sparsify(w_rel, K_REL)
    rw_K = rw_Q.copy()
    vw = sparsify(w_val, K_VAL)
    return idx_qk, idx_v, rw_Q, rw_K, vw, w_qk, w_v


_NC_CACHE = {}


def get_nc(use_f32r=USE_F32R):
    if use_f32r not in _NC_CACHE:
        _NC_CACHE[use_f32r] = build_nc(use_f32r)
    return _NC_CACHE[use_f32r]


def run_device(in_maps, use_f32r=USE_F32R, **kwargs):
    nc = get_nc(use_f32r)
    return run_bass_kernel_spmd(nc, in_maps, list(range(N_CORES)), **kwargs)


def kernel(x, importance, W, b, neuron_emb):
    in_maps = make_in_maps(x, importance, W, b, neuron_emb)
    res = run_device(in_maps)
    wq = np.stack([res.results[i]["wq"] for i in range(B)])       # [B, 1, 512]
    wvr = np.stack([res.results[i]["wvr"] for i in range(B)])     # [B, 3, 416]
    w_qk = np.ascontiguousarray(wq[:, 0, :])                      # [B, 512]
    w_v = np.ascontiguousarray(wvr[:, 0, 0:256])                  # [B, 256]
    w_rel = np.ascontiguousarray(wvr[:, 1, 256:384])              # [B, 128]
    w_val = np.ascontiguousarray(wvr[:, 2, 384:416])              # [B, 32]
    return postprocess(w_qk, w_v, w_rel, w_val)
